# revision 1
# baseline (speedup 1.0000x reference)
"""Bass/Tile kernel for nn_Block_2783138808310 (sparse_attention).

Per-core work (batch-parallel over 8 cores): 2 batches of
  x += attn(LN1(x)); x += deform_attn(LN2(x), ref, value); x += MLP(LN3(x))

Layouts:
  - "n-major": SBUF [128 part = n%128, (nt, feat)], nt = n//128 (8 tiles/batch).
  - "feature-major" (T suffix): SBUF [feat%128 part, (fchunk, n)].
  - Matmuls in bf16 (lhsT/rhs), f32 PSUM accumulate.
Host prep: weights pre-transposed+bf16 (q rows pre-scaled by DH^-0.5), ref
pre-scaled by 32, value pre-transposed, biases packed, deform offset bias
pre-shifted by -1.0 so floor(gx) = rint(gx_shifted + 0.5 - 1.0 ... ) works via
round-nearest f32->i32 casts.
"""
import numpy as np
import concourse.bass as bass
import concourse.tile as tile
import concourse.mybir as mybir
from concourse import bacc
from concourse.masks import make_identity

F32 = mybir.dt.float32
BF16 = mybir.dt.bfloat16
I16 = mybir.dt.int16
I32 = mybir.dt.int32
AF = mybir.ActivationFunctionType
OP = mybir.AluOpType
AX = mybir.AxisListType

B_L, N, C = 2, 1024, 256
NH, DH = 8, 32
NHD, P = 8, 4
HID = 1024
NT = N // 128
KC = C // 128
FPAD = 66
VROWS = N + 2 * FPAD      # 1156
GROWS = VROWS * NHD       # 9248 gather rows; elem [4 corners, 32 d] bf16 = 256B

# pcol column map (packed per-partition constants, f32 [128, 28])
PC_PROJB, PC_VPROJB, PC_OPROJB = 0, 2, 4
PC_FC1B, PC_FC2B = 6, 14
PC_LN1W, PC_LN1B, PC_LN2W, PC_LN2B, PC_LN3W, PC_LN3B = 16, 18, 20, 22, 24, 26


def ap_of(t, offset_elems, dims):
    """AP over tile t: partition dim from t plus free dims [[step, count], ...]."""
    base = t[:] if not isinstance(t, bass.AP) else t
    return bass.AP(tensor=base.tensor, offset=base.offset + offset_elems,
                   ap=[base.ap[0]] + [list(d) for d in dims])


def build(reps: int = 1, debug_names=(), n_cores: int = 8, no_gather: bool = False):
    _nq = 4
    nc = bacc.Bacc("TRN2", target_bir_lowering=False, debug=False, num_devices=n_cores,
                   num_swdge_queues=_nq)

    x_ext = nc.declare_dram_parameter("x", [B_L, N, C], F32, isOutput=False)
    ref32_ext = nc.declare_dram_parameter("ref32", [B_L, N, 2], F32, isOutput=False)
    valT_ext = nc.declare_dram_parameter("valueT", [B_L, C, N], BF16, isOutput=False)
    qkvwT_ext = nc.declare_dram_parameter("qkv_wT", [C, 1152], BF16, isOutput=False)
    projwT_ext = nc.declare_dram_parameter("proj_wT", [C, C], BF16, isOutput=False)
    vprojwT_ext = nc.declare_dram_parameter("vproj_wT", [C, C], BF16, isOutput=False)
    oprojwT_ext = nc.declare_dram_parameter("oproj_wT", [C, C], BF16, isOutput=False)
    offawwT_ext = nc.declare_dram_parameter("offaw_wT", [C, 96], BF16, isOutput=False)
    fc1wT_ext = nc.declare_dram_parameter("fc1_wT", [C, HID], BF16, isOutput=False)
    fc2wT_ext = nc.declare_dram_parameter("fc2_wT", [HID, C], BF16, isOutput=False)
    pcol_ext = nc.declare_dram_parameter("pcol", [128, 28], F32, isOutput=False)
    b96_ext = nc.declare_dram_parameter("b96", [128, 96], F32, isOutput=False)
    hr_ext = nc.declare_dram_parameter("hr", [128, 32], F32, isOutput=False)
    out_ext = nc.declare_dram_parameter("out", [B_L, N, C], F32, isOutput=True)

    dbg_requested = set(debug_names)
    dbg_exts = {}

    from contextlib import ExitStack
    with tile.TileContext(nc) as tc, ExitStack() as stack:
        wp = stack.enter_context(tc.tile_pool(name="wp", bufs=1))
        sp = stack.enter_context(tc.tile_pool(name="sp", bufs=1))
        dp = stack.enter_context(tc.tile_pool(name="dp", bufs=1, space="DRAM"))
        ps_mm = stack.enter_context(tc.tile_pool(name="ps_mm", bufs=2, space="PSUM"))
        ps_att = stack.enter_context(tc.tile_pool(name="ps_att", bufs=2, space="PSUM"))

        def wtile(shape, dtype, tag):
            return wp.tile(shape, dtype, name=tag, tag=tag)

        def stile(shape, dtype, tag, bufs=None):
            return sp.tile(shape, dtype, name=tag, tag=tag, bufs=bufs)

        # ---- weights & constants ----
        def loadw(ext, fdim, kdim=C):
            t = wtile([128, kdim // 128, fdim], BF16, ext.name + "_sb")
            nc.sync.dma_start(t[:], ext.ap().rearrange("(k p) f -> p k f", p=128))
            return t

        qkv_wT = loadw(qkvwT_ext, 1152)
        proj_wT = loadw(projwT_ext, C)
        vproj_wT = loadw(vprojwT_ext, C)
        oproj_wT = loadw(oprojwT_ext, C)
        offaw_wT = loadw(offawwT_ext, 96)
        fc1_wT = loadw(fc1wT_ext, HID)
        fc2_wT = loadw(fc2wT_ext, C, kdim=HID)
        pcol = wtile([128, 28], F32, "pcol_sb")
        nc.sync.dma_start(pcol[:], pcol_ext.ap())
        b96 = wtile([128, 96], F32, "b96_sb")
        nc.sync.dma_start(b96[:], b96_ext.ap())
        hr = wtile([128, 32], F32, "hr_sb")
        nc.sync.dma_start(hr[:], hr_ext.ap())
        ident = wtile([128, 128], BF16, "ident")
        make_identity(nc, ident[:])
        ones1 = wtile([1, 32], BF16, "ones1")
        nc.vector.memset(ones1[:], 1.0)
        ones33 = wtile([33, 32], BF16, "ones33")
        nc.vector.memset(ones33[:], 1.0)
        zpad = wtile([128, 256], BF16, "zpad")
        nc.vector.memset(zpad[:], 0.0)

        vcat = dp.tile([B_L, GROWS, 128], BF16, name="vcat", tag="vcat")

        def dbg(name, src_ap, shape, dtype=F32):
            if name in dbg_requested and name not in dbg_exts:
                e = nc.declare_dram_parameter("dbg_" + name, list(shape), dtype,
                                              isOutput=True)
                dbg_exts[name] = e
                nc.sync.dma_start(e.ap(), src_ap)

        def layernorm(xin, w_col, b_col, ln):
            """n-major [128, NT, C] f32 -> feature-major bf16 [128, KC, N] with affine."""
            stats = stile([128, NT], F32, "ln_mean")
            sq = stile([128, NT], F32, "ln_sq")
            rstd = stile([128, NT], F32, "ln_rstd")
            scr = stile([128, C], F32, "ln_scr")
            y = sp.tile([128, NT, C], BF16, name="ln_y", tag="v2n", bufs=2)
            nc.vector.reduce_sum(stats[:], xin[:], axis=AX.X)
            for nt in range(NT):
                nc.scalar.activation(scr[:], xin[:, nt, :], AF.Square,
                                     accum_out=sq[:, nt:nt + 1])
            nc.vector.tensor_scalar_mul(stats[:], stats[:], 1.0 / C)
            # var = sq/C - m^2 + eps
            nc.vector.tensor_scalar(rstd[:], sq[:], 1.0 / C, 1e-5, OP.mult, OP.add)
            nc.vector.tensor_tensor(sq[:], stats[:], stats[:], op=OP.mult)
            nc.vector.tensor_tensor(rstd[:], rstd[:], sq[:], op=OP.subtract)
            nc.scalar.activation(rstd[:], rstd[:], AF.Ln)
            nc.scalar.activation(rstd[:], rstd[:], AF.Exp, scale=-0.5)
            for nt in range(NT):
                nc.vector.tensor_scalar(y[:, nt, :], xin[:, nt, :],
                                        stats[:, nt:nt + 1], rstd[:, nt:nt + 1],
                                        OP.subtract, OP.mult)
            xT = sp.tile([128, KC, N], BF16, name="ln_xT", tag="ln_xT", bufs=2)
            for cb in range(KC):
                for n4 in range(NT // 4):
                    pt = ps_mm.tile([128, 512], BF16, name="pt_ln", tag="pm")
                    for j in range(4):
                        nt = n4 * 4 + j
                        nc.tensor.transpose(pt[:, j * 128:(j + 1) * 128],
                                            y[:, nt, cb * 128:(cb + 1) * 128], ident[:])
                    nc.scalar.activation(xT[:, cb, n4 * 512:(n4 + 1) * 512], pt[:],
                                         AF.Identity, bias=pcol[:, b_col + cb:b_col + cb + 1],
                                         scale=pcol[:, w_col + cb:w_col + cb + 1])
            return xT

        def matmul_f(xT, wT, fdim, evict, kdim_tiles=KC):
            """out[f, n] = sum_c wT[c, f] xT[c, n]; evict([128,512] psum, m, nch)."""
            for m in range(fdim // 128):
                for nch in range(N // 512):
                    pm = ps_mm.tile([128, 512], F32, name="pm_mm", tag="pm")
                    for kc in range(kdim_tiles):
                        nc.tensor.matmul(pm[:], wT[:, kc, m * 128:(m + 1) * 128],
                                         xT[:, kc, nch * 512:(nch + 1) * 512],
                                         start=(kc == 0), stop=(kc == kdim_tiles - 1))
                    evict(pm, m, nch)

        def transpose_back_add(zT, resid_in, resid_out):
            for n2 in range(NT // 2):
                pt = ps_mm.tile([128, 512], BF16, name="pt_tb", tag="pm")
                for j in range(2):
                    nt = n2 * 2 + j
                    for cb in range(KC):
                        nc.tensor.transpose(pt[:, (j * 2 + cb) * 128:(j * 2 + cb + 1) * 128],
                                            zT[:, cb, nt * 128:(nt + 1) * 128], ident[:])
                nc.vector.tensor_tensor(resid_out[:, n2 * 2, 0:512][:, 0:512] if False else
                                        ap_of(resid_out, n2 * 2 * C, [[1, 512]]),
                                        pt[:], ap_of(resid_in, n2 * 2 * C, [[1, 512]]),
                                        op=OP.add)

        def stage_A(b, st):
            x0 = sp.tile([128, NT, C], F32, name="x0", tag="xres", bufs=3)
            nc.sync.dma_start(x0[:], x_ext.ap()[b].rearrange("(t p) c -> p t c", p=128))
            ref = stile([128, NT, 2], F32, "ref")
            nc.sync.dma_start(ref[:], ref32_ext.ap()[b].rearrange("(t p) c -> p t c", p=128))

            # ================= dense attention =================
            ln1 = layernorm(x0, PC_LN1W, PC_LN1B, "ln1")
            dbg("ln1xT", ln1[:], [128, KC, N], BF16)
            qkvT = stile([128, 9, N], BF16, "qkvT")
            matmul_f(ln1, qkv_wT, 1152,
                     lambda pm, m, nch: nc.vector.tensor_copy(
                         qkvT[:, m, nch * 512:(nch + 1) * 512], pm[:]))

            attn_oT = stile([128, KC, N], BF16, "attn_oT")
            for h in range(NH):
                po = (h % 3) * 32
                qT = qkvT[po:po + 32, 0 + h // 3, :]
                kT = qkvT[po:po + 32, 3 + h // 3, :]
                vT = qkvT[po:po + 32, 6 + h // 3, :]
                v_aug = stile([128, NT, 33], BF16, "v_aug", bufs=2)
                nc.vector.memset(v_aug[:], 1.0)
                for k4 in range(2):
                    pv = ps_mm.tile([128, 128], BF16, name="pv_att", tag="pm")
                    for kk in range(4):
                        kt = k4 * 4 + kk
                        nc.tensor.transpose(pv[0:128, kk * 32:kk * 32 + 32],
                                            vT[:, kt * 128:(kt + 1) * 128],
                                            ident[po:po + 32, po:po + 32])
                    nc.vector.tensor_copy(
                        ap_of(v_aug, k4 * 4 * 33, [[33, 4], [1, 32]]), pv[:])
                psO = ps_att.tile([128, N], F32, name="psO", tag="psO", bufs=2)
                for kt in range(NT):
                    pTk = stile([128, N], BF16, "pTk", bufs=3)
                    for half in range(2):
                        psS = ps_att.tile([128, 512], F32, name="psS", tag="psS", bufs=2)
                        nc.tensor.matmul(psS[:],
                                         kT[:, kt * 128:(kt + 1) * 128],
                                         qT[:, half * 512:(half + 1) * 512],
                                         start=True, stop=True)
                        nc.scalar.activation(pTk[:, half * 512:(half + 1) * 512],
                                             psS[:], AF.Exp)
                    for half in range(2):
                        nc.tensor.matmul(psO[0:33, half * 512:(half + 1) * 512],
                                         v_aug[:, kt, :],
                                         pTk[:, half * 512:(half + 1) * 512],
                                         start=(kt == 0), stop=(kt == NT - 1))
                # rows 0:32 of psO = o; row 32 = sums (ones col last in v_aug)
                rr = stile([33, N], F32, "rr")
                nc.vector.reciprocal(rr[32:33, :], psO[32:33, :])
                rrb = stile([33, N], BF16, "rrb")
                nc.vector.tensor_copy(rrb[32:33, :], rr[32:33, :])
                rr0 = stile([1, N], BF16, "rr0")
                nc.sync.dma_start(rr0[:], rrb[32:33, :])
                Rs = stile([32, N], BF16, "Rs", bufs=2)
                nc.gpsimd.partition_broadcast(Rs[:], rr0[:], channels=32)
                o_n = stile([32, N], BF16, "o_n", bufs=2)
                nc.vector.tensor_tensor(o_n[:], psO[0:32, :], Rs[:], op=OP.mult)
                nc.sync.dma_start(attn_oT[(h % 4) * 32:(h % 4) * 32 + 32, h // 4, :],
                                  o_n[:])
            dbg("attn_oT", attn_oT[:], [128, KC, N], BF16)

            projT = stile([128, KC, N], BF16, "projT")
            matmul_f(attn_oT, proj_wT, C,
                     lambda pm, m, nch: nc.scalar.activation(
                         projT[:, m, nch * 512:(nch + 1) * 512], pm[:], AF.Identity,
                         bias=pcol[:, PC_PROJB + m:PC_PROJB + m + 1]))
            x1 = sp.tile([128, NT, C], F32, name="x1", tag="xres", bufs=3)
            transpose_back_add(projT, x0, x1)
            dbg("x1", x1[:], [128, NT, C])

            st[b]["x0"], st[b]["x1"], st[b]["ref"] = x0, x1, ref

        def stage_V(b, st):
            valT = sp.tile([128, KC, N], BF16, name="valT", tag="qkvT")
            nc.scalar.dma_start(valT[:], valT_ext.ap()[b].rearrange("(k p) n -> p k n", p=128))
            v2T = stile([128, KC, N], BF16, "v2T")
            matmul_f(valT, vproj_wT, C,
                     lambda pm, m, nch: nc.scalar.activation(
                         v2T[:, m, nch * 512:(nch + 1) * 512], pm[:], AF.Identity,
                         bias=pcol[:, PC_VPROJB + m:PC_VPROJB + m + 1]))
            v2n = sp.tile([128, NT, C], BF16, name="v2n", tag="v2n", bufs=2)
            for n2 in range(NT // 2):
                pt = ps_mm.tile([128, 512], BF16, name="pt_v2", tag="pm")
                for j in range(2):
                    nt = n2 * 2 + j
                    for cb in range(KC):
                        nc.tensor.transpose(pt[:, (j * 2 + cb) * 128:(j * 2 + cb + 1) * 128],
                                            v2T[:, cb, nt * 128:(nt + 1) * 128], ident[:])
                nc.vector.tensor_copy(ap_of(v2n, n2 * 2 * C, [[1, 512]]), pt[:])
            dbg("v2n", v2n[:], [128, NT, C], BF16)

            # vcat[b] rows (f, h) = [v2pad[f-66+sh, h, :] for sh in (0,1,32,33)]
            vc = vcat[b]
            for ci, sh in enumerate((0, 1, 32, 33)):
                for nt in range(NT):
                    dst = bass.AP(tensor=vc.tensor,
                                  offset=vc.offset + (66 - sh + nt * 128) * 1024 + ci * 32,
                                  ap=[[1024, 128], [128, NHD], [1, 32]])
                    srcv = ap_of(v2n, nt * C, [[32, NHD], [1, 32]])
                    nc.scalar.dma_start(dst, srcv)
                front, back = 66 - sh, VROWS - (N + 66 - sh)
                if front > 0:
                    dstf = bass.AP(tensor=vc.tensor, offset=vc.offset + ci * 32,
                                   ap=[[1024, front], [128, NHD], [1, 32]])
                    nc.sync.dma_start(dstf, ap_of(zpad[0:front, :], 0, [[32, NHD], [1, 32]]))
                dstb = bass.AP(tensor=vc.tensor,
                               offset=vc.offset + (N + 66 - sh) * 1024 + ci * 32,
                               ap=[[1024, back], [128, NHD], [1, 32]])
                nc.sync.dma_start(dstb, ap_of(zpad[0:back, :], 0, [[32, NHD], [1, 32]]))

        def stage_P(b, st):
            x1, ref = st[b]["x1"], st[b]["ref"]
            # ================= deformable attention =================
            ln2 = layernorm(x1, PC_LN2W, PC_LN2B, "ln2")
            # off/aw n-major
            vc = vcat[b]
            oa = stile([128, NT, 96], F32, "oa")
            for nt in range(NT):
                pm = ps_mm.tile([128, 512], F32, name="pm_oa", tag="pm")
                for kc in range(KC):
                    nc.tensor.matmul(pm[:, 0:96], ln2[:, kc, nt * 128:(nt + 1) * 128],
                                     offaw_wT[:, kc, :], start=(kc == 0), stop=(kc == KC - 1))
                nc.vector.tensor_tensor(oa[:, nt, :], pm[:, 0:96], b96[:], op=OP.add)
            dbg("oa", oa[:], [128, NT, 96])

            # coords ([128, NT, 32] f32 ops; gx here = true_gx - 1.0 via host bias)
            def cf(tag):
                return stile([128, NT, 32], F32, tag)
            gx, gy = cf("gx"), cf("gy")
            nc.vector.tensor_tensor(gx[:], ap_of(oa, 0, [[96, NT], [2, 32]]),
                                    ap_of(ref, 0, [[2, NT], [0, 32]]), op=OP.add)
            nc.vector.tensor_tensor(gy[:], ap_of(oa, 1, [[96, NT], [2, 32]]),
                                    ap_of(ref, 1, [[2, NT], [0, 32]]), op=OP.add)
            # gx here = true_gx - 0.5, so floor(true_gx) = rint(gx) via cast
            x0i = stile([128, NT, 32], I32, "x0i")
            y0i = stile([128, NT, 32], I32, "y0i")
            nc.vector.tensor_copy(x0i[:], gx[:])
            nc.vector.tensor_copy(y0i[:], gy[:])
            x0f, y0f = cf("x0f"), cf("y0f")
            nc.vector.tensor_copy(x0f[:], x0i[:])
            nc.vector.tensor_copy(y0f[:], y0i[:])
            lx, ly = cf("lx"), cf("ly")
            nc.vector.tensor_tensor(lx[:], gx[:], x0f[:], op=OP.subtract)
            nc.vector.tensor_scalar_add(lx[:], lx[:], 0.5)    # lx = true_gx - x0f
            nc.vector.tensor_tensor(ly[:], gy[:], y0f[:], op=OP.subtract)
            nc.vector.tensor_scalar_add(ly[:], ly[:], 0.5)
            vm = {}
            vscr = cf("vm_scr")
            for nm, srcf, lo, hi in (("vx0", x0f, 0.0, 31.0), ("vx1", x0f, -1.0, 30.0),
                                     ("vy0", y0f, 0.0, 31.0), ("vy1", y0f, -1.0, 30.0)):
                m1 = sp.tile([128, NT, 32], F32, name=nm,
                                             tag={"vx0": "gx", "vy0": "gy"}.get(nm, nm))
                nc.vector.tensor_scalar(m1[:], srcf[:], lo, None, OP.is_ge)
                nc.vector.tensor_scalar(vscr[:], srcf[:], hi, None, OP.is_le)
                nc.vector.tensor_tensor(m1[:], m1[:], vscr[:], op=OP.mult)
                vm[nm] = m1
            nc.vector.tensor_scalar(x0f[:], x0f[:], -2.0, 32.0, OP.max, OP.min)
            nc.vector.tensor_scalar(y0f[:], y0f[:], -2.0, 32.0, OP.max, OP.min)
            ridx = cf("ridx")
            nc.vector.tensor_scalar_mul(ridx[:], x0f[:], 8.0)
            nc.vector.tensor_tensor(ridx[:], ridx[:], ap_of(hr, 0, [[0, NT], [1, 32]]),
                                    op=OP.add)
            nc.vector.scalar_tensor_tensor(ridx[:], y0f[:], 256.0, ridx[:],
                                           OP.mult, OP.add)
            ri16 = stile([128, NT, 32], I16, "ri16")
            nc.vector.tensor_copy(ri16[:], ridx[:])
            dbg("ridx", ridx[:], [128, NT, 32])

            xw = stile([128, NT, 64], F32, "xw")
            yw = stile([128, NT, 64], F32, "yw")
            t1 = cf("wscr")
            nc.vector.tensor_scalar(t1[:], lx[:], -1.0, 1.0, OP.mult, OP.add)
            nc.vector.tensor_tensor(ap_of(xw, 0, [[64, NT], [2, 32]]), t1[:],
                                    vm["vx0"][:], op=OP.mult)
            nc.vector.tensor_tensor(ap_of(xw, 1, [[64, NT], [2, 32]]), lx[:],
                                    vm["vx1"][:], op=OP.mult)
            nc.vector.tensor_scalar(t1[:], ly[:], -1.0, 1.0, OP.mult, OP.add)
            nc.vector.tensor_tensor(ap_of(yw, 0, [[64, NT], [2, 32]]), t1[:],
                                    vm["vy0"][:], op=OP.mult)
            nc.vector.tensor_tensor(ap_of(yw, 1, [[64, NT], [2, 32]]), ly[:],
                                    vm["vy1"][:], op=OP.mult)
            awe = stile([128, NT, 32], F32, "awe")
            nc.scalar.activation(awe[:], ap_of(oa, 64, [[96, NT], [1, 32]]), AF.Exp)
            aws = stile([128, NT, NHD], F32, "aws")
            for nt in range(NT):
                nc.vector.reduce_sum(aws[:, nt, :],
                                     ap_of(awe, nt * 32, [[4, NHD], [1, 4]]), axis=AX.X)
            nc.vector.reciprocal(aws[:], aws[:])
            recip = stile([128, NHD, NT], F32, "recip")
            for nt in range(NT):
                nc.vector.tensor_copy(ap_of(recip, nt, [[NT, NHD]]), aws[:, nt, :])
            w_all = stile([128, NHD, NT, 16], F32, "w_all")
            for nt in range(NT):
                wdst = ap_of(w_all, nt * 16, [[NT * 16, NHD], [4, 4], [2, 2], [1, 2]])
                ywap = ap_of(yw, nt * 64, [[8, NHD], [2, 4], [1, 2], [0, 2]])
                xwap = ap_of(xw, nt * 64, [[8, NHD], [2, 4], [0, 2], [1, 2]])
                nc.vector.tensor_tensor(wdst, ywap, xwap, op=OP.mult)
                wflat = ap_of(w_all, nt * 16, [[NT * 16, NHD], [4, 4], [1, 4]])
                aweap = ap_of(awe, nt * 32, [[4, NHD], [1, 4], [0, 4]])
                nc.vector.tensor_tensor(wflat, wflat, aweap, op=OP.mult)
            dbg("w_all", w_all[:], [128, NHD, NT, 16])

            # fold indices into wrapped layout [i%16, h*256 + (nt*4+p)*8 + ni//16]
            # step 1: DVE rearrange (nt,h,p)->(h,nt,p) within partitions
            rs = stile([128, NHD, NT, P], I16, "rs_idx")
            nc.vector.tensor_copy(
                rs[:], ap_of(ri16, 0, [[4, NHD], [32, NT], [1, 4]]))
            # step 2: per-nj DMA, contiguous src -> strided dst (3-dim APs)
            idxs = stile([128, NHD * 256], I16, "idxs")
            for nj in range(8):
                base = rs[nj * 16:(nj + 1) * 16, :, :, :]
                srci = bass.AP(tensor=base.tensor, offset=base.offset,
                               ap=[base.ap[0], [1, 256]])
                dfull = idxs[0:16, :]
                dst = bass.AP(tensor=dfull.tensor, offset=dfull.offset + nj,
                              ap=[dfull.ap[0], [256, NHD], [8, 32]])
                nc.sync.dma_start(dst, srci)
            for g in (16, 32, 64):
                nc.sync.dma_start(idxs[g:2 * g, :], idxs[0:g, :])
            dbg("idxs", idxs[:], [128, NHD * 256], I16)
            if "vcat0" in dbg_requested and "vcat0" not in dbg_exts:
                e = nc.declare_dram_parameter("dbg_vcat0", [GROWS, 128], BF16,
                                              isOutput=True)
                dbg_exts["vcat0"] = e
                nc.sync.dma_start(e.ap(), bass.AP(tensor=vc.tensor, offset=vc.offset,
                                                  ap=[[128, GROWS], [1, 128]]))

            st[b]["w_all"], st[b]["recip"], st[b]["idxs"] = w_all, recip, idxs

        def stage_G(b, st):
            w_all, recip, idxs = st[b]["w_all"], st[b]["recip"], st[b]["idxs"]
            vc = vcat[b]
            # gather + combine per h
            dfo = sp.tile([128, NT, C], BF16, name="dfo", tag="v2n", bufs=2)
            for h in range(NHD):
                G = stile([128, 32, 128], BF16, "G", bufs=3)
                if no_gather:
                    nc.vector.memset(G[:], 0.0)
                else:
                    nc.gpsimd.dma_gather(G[:],
                                         bass.AP(tensor=vc.tensor, offset=vc.offset,
                                                 ap=[[128, GROWS], [1, 128]]),
                                         idxs[:, h * 256:(h + 1) * 256], num_idxs=4096,
                                         num_idxs_reg=4096, elem_size=128,
                                         single_packet=False,
                                         queue_num=h % _nq)
                T = stile([128, 32, 4, 32], BF16, "T")
                mul_eng = nc.vector
                mul_eng.tensor_tensor(
                    T[:], ap_of(G, 0, [[128, 32], [32, 4], [1, 32]]),
                    ap_of(w_all, h * NT * 16, [[4, 32], [1, 4], [0, 32]]), op=OP.mult)
                tc2 = sp.tile([128, 32, 2, 32], BF16, name="tc2", tag="G", bufs=3)
                nc.vector.tensor_tensor(tc2[:, :, 0, :], T[:, :, 0, :], T[:, :, 2, :],
                                        op=OP.add)
                nc.vector.tensor_tensor(tc2[:, :, 1, :], T[:, :, 1, :], T[:, :, 3, :],
                                        op=OP.add)
                t2 = stile([128, 32, 32], BF16, "t2")
                nc.vector.tensor_tensor(t2[:], tc2[:, :, 0, :], tc2[:, :, 1, :], op=OP.add)
                o1 = stile([128, NT, 2, 32], BF16, "o1")
                nc.vector.tensor_tensor(o1[:], ap_of(t2, 0, [[128, NT], [32, 2], [1, 32]]),
                                        ap_of(t2, 64, [[128, NT], [32, 2], [1, 32]]),
                                        op=OP.add)
                nc.vector.tensor_tensor(o1[:, :, 0, :], o1[:, :, 0, :], o1[:, :, 1, :],
                                        op=OP.add)
                nc.vector.tensor_tensor(ap_of(dfo, h * DH, [[C, NT], [1, DH]]),
                                        o1[:, :, 0, :],
                                        ap_of(recip, h * NT, [[1, NT], [0, 32]]),
                                        op=OP.mult)
            dbg("dfo", dfo[:], [128, NT, C], BF16)

            st[b]["dfo"] = dfo

        def stage_C(b, st):
            dfo, x1 = st[b]["dfo"], st[b]["x1"]
            dfoT = stile([128, KC, N], BF16, "dfoT")
            for cb in range(KC):
                for n2 in range(NT // 2):
                    pt = ps_mm.tile([128, 256], BF16, name="pt_df", tag="pm")
                    for j in range(2):
                        nt = n2 * 2 + j
                        nc.tensor.transpose(pt[:, j * 128:(j + 1) * 128],
                                            dfo[:, nt, cb * 128:(cb + 1) * 128], ident[:])
                    nc.vector.tensor_copy(dfoT[:, cb, n2 * 256:(n2 + 1) * 256], pt[:])
            oprojT = stile([128, KC, N], BF16, "oprojT")
            matmul_f(dfoT, oproj_wT, C,
                     lambda pm, m, nch: nc.scalar.activation(
                         oprojT[:, m, nch * 512:(nch + 1) * 512], pm[:], AF.Identity,
                         bias=pcol[:, PC_OPROJB + m:PC_OPROJB + m + 1]))
            x2 = sp.tile([128, NT, C], F32, name="x2", tag="xres", bufs=3)
            transpose_back_add(oprojT, x1, x2)
            dbg("x2", x2[:], [128, NT, C])

            # ================= MLP =================
            ln3 = layernorm(x2, PC_LN3W, PC_LN3B, "ln3")
            hT = stile([128, HID // 128, N], BF16, "hT")
            matmul_f(ln3, fc1_wT, HID,
                     lambda pm, m, nch: nc.scalar.activation(
                         hT[:, m, nch * 512:(nch + 1) * 512], pm[:], AF.Gelu,
                         bias=pcol[:, PC_FC1B + m:PC_FC1B + m + 1]))
            f2T = stile([128, KC, N], BF16, "f2T")
            matmul_f(hT, fc2_wT, C,
                     lambda pm, m, nch: nc.scalar.activation(
                         f2T[:, m, nch * 512:(nch + 1) * 512], pm[:], AF.Identity,
                         bias=pcol[:, PC_FC2B + m:PC_FC2B + m + 1]),
                     kdim_tiles=HID // 128)
            x3 = sp.tile([128, NT, C], F32, name="x3", tag="xres", bufs=3)
            transpose_back_add(f2T, x2, x3)

            nc.sync.dma_start(out_ext.ap()[b].rearrange("(t p) c -> p t c", p=128), x3[:])

        def all_bodies():
            st = [{}, {}]
            stage_A(0, st)
            stage_V(0, st)
            stage_P(0, st)
            stage_A(1, st)
            stage_V(1, st)
            stage_G(0, st)
            stage_P(1, st)
            stage_G(1, st)
            stage_C(0, st)
            stage_C(1, st)

        if reps == 1:
            all_bodies()
        elif reps < 0:
            for rep in range(-reps):
                all_bodies()
        else:
            with tc.For_i(0, reps):
                all_bodies()

    nc.compile()
    return nc, dbg_exts


def host_prep(inputs, n_cores=8):
    """Preprocess FULL inputs -> list of per-core in_maps (B=16 -> 2 per core)."""
    import ml_dtypes
    bf16 = ml_dtypes.bfloat16
    f32 = np.float32
    g = {k: np.asarray(v) for k, v in inputs.items()}
    DHs = 1.0 / np.sqrt(DH)

    qkv_w = g["qkv_w"].astype(f32)                     # [3C, C] rows (qkv, h, d)
    qkv_wT = np.zeros((C, 1152), f32)   # 9 chunks: q 0-2, k 3-5, v 6-8; po=(h%3)*32
    for t3 in range(3):
        for h in range(8):
            rows = qkv_w[t3 * C + h * DH: t3 * C + (h + 1) * DH]   # [32, C]
            col = (t3 * 3 + h // 3) * 128 + (h % 3) * 32
            qkv_wT[:, col:col + DH] = rows.T * (DHs if t3 == 0 else 1.0)
    shared = {
        "qkv_wT": qkv_wT.astype(bf16),
        "proj_wT": g["proj_w"].T.astype(bf16).copy(),
        "vproj_wT": g["vproj_w"].T.astype(bf16).copy(),
        "oproj_wT": g["oproj_w"].T.astype(bf16).copy(),
        "offaw_wT": np.concatenate([g["off_w"], g["aw_w"]], 0).T.astype(bf16).copy(),
        "fc1_wT": g["fc1_w"].T.astype(bf16).copy(),
        "fc2_wT": g["fc2_w"].T.astype(bf16).copy(),
    }
    pcol = np.zeros((128, 28), f32)
    for col, v in ((PC_PROJB, g["proj_b"]), (PC_VPROJB, g["vproj_b"]),
                   (PC_OPROJB, g["oproj_b"]), (PC_FC1B, g["fc1_b"]),
                   (PC_FC2B, g["fc2_b"]), (PC_LN1W, g["ln1_w"]), (PC_LN1B, g["ln1_b"]),
                   (PC_LN2W, g["ln2_w"]), (PC_LN2B, g["ln2_b"]),
                   (PC_LN3W, g["ln3_w"]), (PC_LN3B, g["ln3_b"])):
        v = np.asarray(v, f32)
        pcol[:, col:col + v.size // 128] = v.reshape(-1, 128).T
    shared["pcol"] = pcol
    b96 = np.zeros((128, 96), f32)
    b96[:, 0:64] = g["off_b"].astype(f32) - 1.0        # -0.5 (grid) -0.5 (rint floor)
    b96[:, 64:96] = g["aw_b"].astype(f32)
    shared["b96"] = b96
    hr = np.zeros((128, 32), f32)
    for h in range(NHD):
        for p in range(P):
            hr[:, h * 4 + p] = h + (FPAD - 2112) * 8   # r = y0*256 + x0*8 + 8*(66-2112)+h
    # r = f*8 + h, f = y0*32 + x0 + 66 (y0, x0 already include no shift)
    # => r = y0*256 + x0*8 + 528 + h
    hr[:] = 0.0
    for h in range(NHD):
        for p in range(P):
            hr[:, h * 4 + p] = h + 528.0
    shared["hr"] = hr

    bpc = g["x"].shape[0] // n_cores
    assert bpc == B_L
    maps = []
    for c in range(n_cores):
        sl = slice(c * bpc, (c + 1) * bpc)
        m = dict(shared)
        m["x"] = g["x"][sl].astype(f32)
        m["ref32"] = (g["ref"][sl] * 32.0).astype(f32)
        m["valueT"] = np.ascontiguousarray(
            g["value"][sl].transpose(0, 2, 1)).astype(bf16)
        maps.append(m)
    return maps


_BUILD_CACHE = {}


def kernel(**inputs):
    """Full-input entry point: shards batch across 8 NeuronCores, runs the
    Bass kernel, gathers the full [16, 1024, 256] output."""
    from concourse.bass_utils import run_bass_kernel_spmd
    key = "k"
    if key not in _BUILD_CACHE:
        _BUILD_CACHE[key] = build(reps=1)[0]
    nc = _BUILD_CACHE[key]
    maps = host_prep(inputs, 8)
    res = run_bass_kernel_spmd(nc, maps, core_ids=list(range(8)))
    out = np.concatenate([res.results[c]["out"] for c in range(8)], axis=0)
    return out.astype(np.float32)



# revision 12
# speedup vs baseline: 2.3463x; 2.3463x over previous
"""Bass/Tile kernel for nn_Block_2783138808310 (sparse_attention).

Per-core work (batch-parallel over 8 cores): 2 batches of
  x += attn(LN1(x)); x += deform_attn(LN2(x), ref, value); x += MLP(LN3(x))

Layouts:
  - "n-major": SBUF [128 part = n%128, (nt, feat)], nt = n//128 (8 tiles/batch).
  - "feature-major" (T suffix): SBUF [feat%128 part, (fchunk, n)].
  - Matmuls in bf16 (lhsT/rhs), f32 PSUM accumulate.
  - attention heads packed 4/chunk: feat = (h//4)*128 + (h%4)*32 + d, so
    K=32 score matmuls row-tile 4-up and M=32 AV matmuls col-tile 4-up.
Host prep: weights pre-transposed+bf16 (q rows pre-scaled by DH^-0.5), ref
pre-scaled by 32, value pre-transposed, biases packed, deform offset bias
pre-shifted by -1.0 so floor(gx) works via round-nearest f32->i32 casts.
"""
import numpy as np
import concourse.bass as bass
import concourse.tile as tile
import concourse.mybir as mybir
from concourse import bacc
from concourse.masks import make_identity

F32 = mybir.dt.float32
BF16 = mybir.dt.bfloat16
I16 = mybir.dt.int16
I32 = mybir.dt.int32
AF = mybir.ActivationFunctionType
OP = mybir.AluOpType
AX = mybir.AxisListType

B_L, N, C = 2, 1024, 256
NH, DH = 8, 32
NHD, P = 8, 4
HID = 1024
NT = N // 128
KC = C // 128
FPAD = 66
VROWS = N + 2 * FPAD      # 1156
GROWS = VROWS * NHD       # 9248 gather rows; elem [4 corners, 32 d] bf16 = 256B

# pcol column map (packed per-partition constants, f32 [128, 28])
PC_PROJB, PC_VPROJB, PC_OPROJB = 0, 2, 4
PC_FC1B, PC_FC2B = 6, 14
PC_LN1W, PC_LN1B, PC_LN2W, PC_LN2B, PC_LN3W, PC_LN3B = 16, 18, 20, 22, 24, 26


def ap_of(t, offset_elems, dims):
    """AP over tile t: partition dim from t plus free dims [[step, count], ...]."""
    base = t[:] if not isinstance(t, bass.AP) else t
    return bass.AP(tensor=base.tensor, offset=base.offset + offset_elems,
                   ap=[base.ap[0]] + [list(d) for d in dims])


def build(reps: int = 1, debug_names=(), n_cores: int = 8, no_gather: bool = False):
    _nq = 4
    nc = bacc.Bacc("TRN2", target_bir_lowering=False, debug=False, num_devices=n_cores,
                   num_swdge_queues=_nq)

    x_ext = nc.declare_dram_parameter("x", [B_L, N, C], F32, isOutput=False)
    ref32_ext = nc.declare_dram_parameter("ref32", [B_L, N, 2], F32, isOutput=False)
    valT_ext = nc.declare_dram_parameter("valueT", [B_L, C, N], BF16, isOutput=False)
    qkvwT_ext = nc.declare_dram_parameter("qkv_wT", [C, 768], BF16, isOutput=False)
    projwT_ext = nc.declare_dram_parameter("proj_wT", [C, C], BF16, isOutput=False)
    vprojwT_ext = nc.declare_dram_parameter("vproj_wT", [C, C], BF16, isOutput=False)
    oprojwT_ext = nc.declare_dram_parameter("oproj_wT", [C, C], BF16, isOutput=False)
    offawwT_ext = nc.declare_dram_parameter("offaw_wT", [C, 96], BF16, isOutput=False)
    fc1wT_ext = nc.declare_dram_parameter("fc1_wT", [C, HID], BF16, isOutput=False)
    fc2wT_ext = nc.declare_dram_parameter("fc2_wT", [HID, C], BF16, isOutput=False)
    pcol_ext = nc.declare_dram_parameter("pcol", [128, 28], F32, isOutput=False)
    b96_ext = nc.declare_dram_parameter("b96", [128, 96], F32, isOutput=False)
    hr_ext = nc.declare_dram_parameter("hr", [128, 32], F32, isOutput=False)
    out_ext = nc.declare_dram_parameter("out", [B_L, N, C], F32, isOutput=True)

    dbg_requested = set(debug_names)
    dbg_exts = {}

    from contextlib import ExitStack
    with tile.TileContext(nc) as tc, ExitStack() as stack:
        wp = stack.enter_context(tc.tile_pool(name="wp", bufs=1))
        sp = stack.enter_context(tc.tile_pool(name="sp", bufs=1))
        dp = stack.enter_context(tc.tile_pool(name="dp", bufs=1, space="DRAM"))
        ps_mm = stack.enter_context(tc.tile_pool(name="ps_mm", bufs=2, space="PSUM"))
        ps_att = stack.enter_context(tc.tile_pool(name="ps_att", bufs=2, space="PSUM"))

        def wtile(shape, dtype, tag):
            return wp.tile(shape, dtype, name=tag, tag=tag)

        def stile(shape, dtype, tag, bufs=None):
            return sp.tile(shape, dtype, name=tag, tag=tag, bufs=bufs)

        # ---- weights & constants ----
        def loadw(ext, fdim, kdim=C):
            t = wtile([128, kdim // 128, fdim], BF16, ext.name + "_sb")
            nc.sync.dma_start(t[:], ext.ap().rearrange("(k p) f -> p k f", p=128))
            return t

        qkv_wT = loadw(qkvwT_ext, 768)
        proj_wT = loadw(projwT_ext, C)
        vproj_wT = loadw(vprojwT_ext, C)
        oproj_wT = loadw(oprojwT_ext, C)
        offaw_wT = loadw(offawwT_ext, 96)
        fc1_wT = loadw(fc1wT_ext, HID)
        fc2_wT = loadw(fc2wT_ext, C, kdim=HID)
        pcol = wtile([128, 28], F32, "pcol_sb")
        nc.sync.dma_start(pcol[:], pcol_ext.ap())
        b96 = wtile([128, 96], F32, "b96_sb")
        nc.sync.dma_start(b96[:], b96_ext.ap())
        hr = wtile([128, 32], F32, "hr_sb")
        nc.sync.dma_start(hr[:], hr_ext.ap())
        ident = wtile([128, 128], BF16, "ident")
        make_identity(nc, ident[:])
        onesb = wtile([128, 32], BF16, "onesb")
        nc.vector.memset(onesb[:], 1.0)
        zpad = wtile([128, 256], BF16, "zpad")
        nc.vector.memset(zpad[:], 0.0)

        vcat = dp.tile([B_L, GROWS, 128], BF16, name="vcat", tag="vcat")

        def dbg(name, src_ap, shape, dtype=F32):
            if name in dbg_requested and name not in dbg_exts:
                e = nc.declare_dram_parameter("dbg_" + name, list(shape), dtype,
                                              isOutput=True)
                dbg_exts[name] = e
                nc.sync.dma_start(e.ap(), src_ap)

        def layernorm(xin, w_col, b_col, ln):
            """n-major [128, NT, C] f32 -> feature-major bf16 [128, KC, N] with affine."""
            stats = stile([128, NT], F32, "ln_mean")
            sq = stile([128, NT], F32, "ln_sq")
            rstd = stile([128, NT], F32, "ln_rstd")
            scr = stile([128, C], F32, "ln_scr")
            y = sp.tile([128, NT, C], BF16, name="ln_y", tag="v2n", bufs=2)
            nc.vector.reduce_sum(stats[:], xin[:], axis=AX.X)
            for nt in range(NT):
                nc.scalar.activation(scr[:], xin[:, nt, :], AF.Square,
                                     accum_out=sq[:, nt:nt + 1])
            nc.vector.tensor_scalar_mul(stats[:], stats[:], 1.0 / C)
            # var = sq/C - m^2 + eps
            nc.vector.tensor_scalar(rstd[:], sq[:], 1.0 / C, 1e-5, OP.mult, OP.add)
            nc.vector.tensor_tensor(sq[:], stats[:], stats[:], op=OP.mult)
            nc.vector.tensor_tensor(rstd[:], rstd[:], sq[:], op=OP.subtract)
            nc.scalar.activation(rstd[:], rstd[:], AF.Ln)
            nc.scalar.activation(rstd[:], rstd[:], AF.Exp, scale=-0.5)
            for nt in range(NT):
                nc.vector.tensor_scalar(y[:, nt, :], xin[:, nt, :],
                                        stats[:, nt:nt + 1], rstd[:, nt:nt + 1],
                                        OP.subtract, OP.mult)
            xT = sp.tile([128, KC, N], BF16, name="ln_xT", tag="ln_xT", bufs=2)
            for cb in range(KC):
                for n4 in range(NT // 4):
                    pt = ps_mm.tile([128, 512], BF16, name="pt_ln", tag="pm")
                    for j in range(4):
                        nt = n4 * 4 + j
                        nc.tensor.transpose(pt[:, j * 128:(j + 1) * 128],
                                            y[:, nt, cb * 128:(cb + 1) * 128], ident[:])
                    nc.scalar.activation(xT[:, cb, n4 * 512:(n4 + 1) * 512], pt[:],
                                         AF.Identity, bias=pcol[:, b_col + cb:b_col + cb + 1],
                                         scale=pcol[:, w_col + cb:w_col + cb + 1])
            return xT

        def matmul_f(xT, wT, fdim, evict, kdim_tiles=KC):
            """out[f, n] = sum_c wT[c, f] xT[c, n]; evict([128,512] psum, m, nch)."""
            for m in range(fdim // 128):
                for nch in range(N // 512):
                    pm = ps_mm.tile([128, 512], F32, name="pm_mm", tag="pm")
                    for kc in range(kdim_tiles):
                        nc.tensor.matmul(pm[:], wT[:, kc, m * 128:(m + 1) * 128],
                                         xT[:, kc, nch * 512:(nch + 1) * 512],
                                         start=(kc == 0), stop=(kc == kdim_tiles - 1))
                    evict(pm, m, nch)

        def transpose_back_add(zT, resid_in, resid_out):
            for n2 in range(NT // 2):
                pt = ps_mm.tile([128, 512], BF16, name="pt_tb", tag="pm")
                for j in range(2):
                    nt = n2 * 2 + j
                    for cb in range(KC):
                        nc.tensor.transpose(pt[:, (j * 2 + cb) * 128:(j * 2 + cb + 1) * 128],
                                            zT[:, cb, nt * 128:(nt + 1) * 128], ident[:])
                nc.vector.tensor_tensor(ap_of(resid_out, n2 * 2 * C, [[1, 512]]),
                                        pt[:], ap_of(resid_in, n2 * 2 * C, [[1, 512]]),
                                        op=OP.add)

        def stage_L(b, st):
            x0 = sp.tile([128, NT, C], F32, name="x0", tag="xres", bufs=4)
            nc.sync.dma_start(x0[:], x_ext.ap()[b].rearrange("(t p) c -> p t c", p=128))
            ref = stile([128, NT, 2], F32, "ref", bufs=2)
            nc.sync.dma_start(ref[:], ref32_ext.ap()[b].rearrange("(t p) c -> p t c", p=128))
            st[b]["x0"], st[b]["ref"] = x0, ref

        def stage_A(b, st):
            x0, ref = st[b]["x0"], st[b]["ref"]

            # ================= dense attention =================
            ln1 = layernorm(x0, PC_LN1W, PC_LN1B, "ln1")
            dbg("ln1xT", ln1[:], [128, KC, N], BF16)
            qkvT = stile([128, 6, N], BF16, "qkvT", bufs=1)
            matmul_f(ln1, qkv_wT, 768,
                     lambda pm, m, nch: nc.vector.tensor_copy(
                         qkvT[:, m, nch * 512:(nch + 1) * 512], pm[:]))

            # v chunks -> n-major v_n[g]: [128 k%128, g, kt, (h'*32+d)]
            v_n = stile([128, 2, NT, 128], BF16, "v_n", bufs=1)
            for g in range(2):
                for k2 in range(NT // 2):
                    pt = ps_mm.tile([128, 256], BF16, name="pt_vn", tag="pm")
                    for j in range(2):
                        kt = k2 * 2 + j
                        nc.tensor.transpose(pt[:, j * 128:(j + 1) * 128],
                                            qkvT[:, 4 + g, kt * 128:(kt + 1) * 128],
                                            ident[:])
                    nc.vector.tensor_copy(
                        ap_of(v_n, (g * NT + k2 * 2) * 128, [[1, 256]]), pt[:])

            # scores (4-up row-tiled K=32) -> exp -> AV (4-up col-tiled M=32)
            oT_raw = stile([128, 2, 2, 512], BF16, "oT_raw")   # [*, g, half, q]
            rb = stile([128, 4, 512], BF16, "rb")              # 1/den, bcast over d
            for g in range(2):
                for half in range(2):
                    psO = ps_att.tile([128, 512], F32, name="psO", tag="psO", bufs=1)
                    psD = ps_att.tile([128, 512], F32, name="psD", tag="psD", bufs=1)
                    for kt in range(NT):
                        quad = ps_att.tile([128, 4, 512], F32, name="quad",
                                           tag="quad", bufs=1)
                        for hp in range(4):
                            nc.tensor.matmul(
                                quad[:, hp, :],
                                qkvT[hp * 32:hp * 32 + 32, 2 + g,
                                     kt * 128:(kt + 1) * 128],
                                qkvT[hp * 32:hp * 32 + 32, 0 + g,
                                     half * 512:(half + 1) * 512],
                                start=True, stop=True, tile_position=(hp * 32, 0))
                        pTk = stile([128, 4, 512], BF16, "pTk", bufs=2)
                        nc.scalar.activation(pTk[:], quad[:], AF.Exp)
                        for hp in range(4):
                            nc.tensor.matmul(
                                psO[hp * 32:(hp + 1) * 32, :],
                                v_n[:, g, kt, hp * 32:(hp + 1) * 32],
                                pTk[:, hp, :],
                                start=(kt == 0), stop=(kt == NT - 1),
                                tile_position=(0, hp * 32))
                        for hp in range(4):
                            nc.tensor.matmul(
                                psD[hp * 32:(hp + 1) * 32, :],
                                onesb[:], pTk[:, hp, :],
                                start=(kt == 0), stop=(kt == NT - 1),
                                tile_position=(0, hp * 32))
                    nc.vector.tensor_copy(oT_raw[:, g, half, :], psO[:])
                    with nc.allow_low_precision(reason="softmax denom, bf16 ok"):
                        nc.vector.reciprocal(rb[:, g * 2 + half, :], psD[:])
            attn_oT = sp.tile([128, KC, N], BF16, name="attn_oT", tag="v2T", bufs=2)
            nc.vector.tensor_tensor(attn_oT[:], ap_of(oT_raw, 0, [[1, 2048]]),
                                    rb[:], op=OP.mult)
            dbg("oT_raw", oT_raw[:], [128, 2, 2, 512], BF16)
            dbg("rb", rb[:], [128, 4, 512], BF16)
            dbg("attn_oT", attn_oT[:], [128, KC, N], BF16)

            projT = stile([128, KC, N], BF16, "projT", bufs=2)
            matmul_f(attn_oT, proj_wT, C,
                     lambda pm, m, nch: nc.scalar.activation(
                         projT[:, m, nch * 512:(nch + 1) * 512], pm[:], AF.Identity,
                         bias=pcol[:, PC_PROJB + m:PC_PROJB + m + 1]))
            x1 = sp.tile([128, NT, C], F32, name="x1", tag="xres", bufs=4)
            transpose_back_add(projT, x0, x1)
            dbg("x1", x1[:], [128, NT, C])

            st[b]["x1"] = x1

        def stage_V(b, st):
            valT = sp.tile([128, KC, N], BF16, name="valT", tag="valT", bufs=1)
            nc.scalar.dma_start(valT[:], valT_ext.ap()[b].rearrange("(k p) n -> p k n", p=128))
            v2T = stile([128, KC, N], BF16, "v2T", bufs=2)
            matmul_f(valT, vproj_wT, C,
                     lambda pm, m, nch: nc.scalar.activation(
                         v2T[:, m, nch * 512:(nch + 1) * 512], pm[:], AF.Identity,
                         bias=pcol[:, PC_VPROJB + m:PC_VPROJB + m + 1]))
            v2n = sp.tile([128, NT, C], BF16, name="v2n", tag="v2n", bufs=2)
            for n2 in range(NT // 2):
                pt = ps_mm.tile([128, 512], BF16, name="pt_v2", tag="pm")
                for j in range(2):
                    nt = n2 * 2 + j
                    for cb in range(KC):
                        nc.tensor.transpose(pt[:, (j * 2 + cb) * 128:(j * 2 + cb + 1) * 128],
                                            v2T[:, cb, nt * 128:(nt + 1) * 128], ident[:])
                nc.vector.tensor_copy(ap_of(v2n, n2 * 2 * C, [[1, 512]]), pt[:])
            dbg("v2n", v2n[:], [128, NT, C], BF16)

            # vcat[b] rows (f, h) = [v2pad[f-66+sh, h, :] for sh in (0,1,32,33)]
            vc = vcat[b]
            for ci, sh in enumerate((0, 1, 32, 33)):
                for nt in range(NT):
                    dst = bass.AP(tensor=vc.tensor,
                                  offset=vc.offset + (66 - sh + nt * 128) * 1024 + ci * 32,
                                  ap=[[1024, 128], [128, NHD], [1, 32]])
                    srcv = ap_of(v2n, nt * C, [[32, NHD], [1, 32]])
                    nc.scalar.dma_start(dst, srcv)
                front, back = 66 - sh, VROWS - (N + 66 - sh)
                if front > 0:
                    dstf = bass.AP(tensor=vc.tensor, offset=vc.offset + ci * 32,
                                   ap=[[1024, front], [128, NHD], [1, 32]])
                    nc.sync.dma_start(dstf, ap_of(zpad[0:front, :], 0, [[32, NHD], [1, 32]]))
                dstb = bass.AP(tensor=vc.tensor,
                               offset=vc.offset + (N + 66 - sh) * 1024 + ci * 32,
                               ap=[[1024, back], [128, NHD], [1, 32]])
                nc.sync.dma_start(dstb, ap_of(zpad[0:back, :], 0, [[32, NHD], [1, 32]]))

        def stage_P(b, st):
            x1, ref = st[b]["x1"], st[b]["ref"]
            # ================= deformable attention =================
            ln2 = layernorm(x1, PC_LN2W, PC_LN2B, "ln2")
            oa = stile([128, NT, 96], F32, "oa")
            for nt in range(NT):
                pm = ps_mm.tile([128, 512], F32, name="pm_oa", tag="pm")
                for kc in range(KC):
                    nc.tensor.matmul(pm[:, 0:96], ln2[:, kc, nt * 128:(nt + 1) * 128],
                                     offaw_wT[:, kc, :], start=(kc == 0), stop=(kc == KC - 1))
                nc.vector.tensor_tensor(oa[:, nt, :], pm[:, 0:96], b96[:], op=OP.add)
            dbg("oa", oa[:], [128, NT, 96])

            # coords ([128, NT, 32] f32 ops; gx here = true_gx - 1.0 via host bias)
            def cf(tag):
                return stile([128, NT, 32], F32, tag)
            gx, gy = cf("gx"), cf("gy")
            nc.vector.tensor_tensor(gx[:], ap_of(oa, 0, [[96, NT], [2, 32]]),
                                    ap_of(ref, 0, [[2, NT], [0, 32]]), op=OP.add)
            nc.vector.tensor_tensor(gy[:], ap_of(oa, 1, [[96, NT], [2, 32]]),
                                    ap_of(ref, 1, [[2, NT], [0, 32]]), op=OP.add)
            # gx here = true_gx - 0.5, so floor(true_gx) = rint(gx) via cast
            x0i = stile([128, NT, 32], I32, "x0i")
            y0i = sp.tile([128, NT, 32], I32, name="y0i", tag="x0i", bufs=1)
            nc.vector.tensor_copy(x0i[:], gx[:])
            nc.vector.tensor_copy(y0i[:], gy[:])
            x0f, y0f = cf("x0f"), cf("y0f")
            nc.vector.tensor_copy(x0f[:], x0i[:])
            nc.vector.tensor_copy(y0f[:], y0i[:])
            lx, ly = cf("lx"), cf("ly")
            nc.vector.tensor_tensor(lx[:], gx[:], x0f[:], op=OP.subtract)
            nc.vector.tensor_scalar_add(lx[:], lx[:], 0.5)    # lx = true_gx - x0f
            nc.vector.tensor_tensor(ly[:], gy[:], y0f[:], op=OP.subtract)
            nc.vector.tensor_scalar_add(ly[:], ly[:], 0.5)
            vm = {}
            vscr = cf("vm_scr")
            for nm, srcf, lo, hi in (("vx0", x0f, 0.0, 31.0), ("vx1", x0f, -1.0, 30.0),
                                     ("vy0", y0f, 0.0, 31.0), ("vy1", y0f, -1.0, 30.0)):
                m1 = sp.tile([128, NT, 32], F32, name=nm,
                                             tag={"vx0": "gx", "vy0": "gy"}.get(nm, nm))
                nc.vector.tensor_scalar(m1[:], srcf[:], lo, None, OP.is_ge)
                nc.vector.tensor_scalar(vscr[:], srcf[:], hi, None, OP.is_le)
                nc.vector.tensor_tensor(m1[:], m1[:], vscr[:], op=OP.mult)
                vm[nm] = m1
            nc.vector.tensor_scalar(x0f[:], x0f[:], -2.0, 32.0, OP.max, OP.min)
            nc.vector.tensor_scalar(y0f[:], y0f[:], -2.0, 32.0, OP.max, OP.min)
            ridx = cf("ridx")
            nc.vector.tensor_scalar_mul(ridx[:], x0f[:], 8.0)
            nc.vector.tensor_tensor(ridx[:], ridx[:], ap_of(hr, 0, [[0, NT], [1, 32]]),
                                    op=OP.add)
            nc.vector.scalar_tensor_tensor(ridx[:], y0f[:], 256.0, ridx[:],
                                           OP.mult, OP.add)
            ri16 = stile([128, NT, 32], I16, "ri16")
            nc.vector.tensor_copy(ri16[:], ridx[:])
            dbg("ridx", ridx[:], [128, NT, 32])

            xw = stile([128, NT, 64], F32, "xw")
            yw = stile([128, NT, 64], F32, "yw")
            t1 = cf("wscr")
            nc.vector.tensor_scalar(t1[:], lx[:], -1.0, 1.0, OP.mult, OP.add)
            nc.vector.tensor_tensor(ap_of(xw, 0, [[64, NT], [2, 32]]), t1[:],
                                    vm["vx0"][:], op=OP.mult)
            nc.vector.tensor_tensor(ap_of(xw, 1, [[64, NT], [2, 32]]), lx[:],
                                    vm["vx1"][:], op=OP.mult)
            nc.vector.tensor_scalar(t1[:], ly[:], -1.0, 1.0, OP.mult, OP.add)
            nc.vector.tensor_tensor(ap_of(yw, 0, [[64, NT], [2, 32]]), t1[:],
                                    vm["vy0"][:], op=OP.mult)
            nc.vector.tensor_tensor(ap_of(yw, 1, [[64, NT], [2, 32]]), ly[:],
                                    vm["vy1"][:], op=OP.mult)
            awe = stile([128, NT, 32], F32, "awe")
            nc.scalar.activation(awe[:], ap_of(oa, 64, [[96, NT], [1, 32]]), AF.Exp)
            aws = stile([128, NT, NHD], F32, "aws")
            for nt in range(NT):
                nc.vector.reduce_sum(aws[:, nt, :],
                                     ap_of(awe, nt * 32, [[4, NHD], [1, 4]]), axis=AX.X)
            nc.vector.reciprocal(aws[:], aws[:])
            w_all = stile([128, NHD, NT, 16], F32, "w_all")
            for nt in range(NT):
                wdst = ap_of(w_all, nt * 16, [[NT * 16, NHD], [4, 4], [2, 2], [1, 2]])
                ywap = ap_of(yw, nt * 64, [[8, NHD], [2, 4], [1, 2], [0, 2]])
                xwap = ap_of(xw, nt * 64, [[8, NHD], [2, 4], [0, 2], [1, 2]])
                nc.vector.tensor_tensor(wdst, ywap, xwap, op=OP.mult)
                wflat = ap_of(w_all, nt * 16, [[NT * 16, NHD], [4, 4], [1, 4]])
                aweap = ap_of(awe, nt * 32, [[4, NHD], [1, 4], [0, 4]])
                nc.vector.tensor_tensor(wflat, wflat, aweap, op=OP.mult)
                # fold softmax 1/sum into the weights
                rcap = ap_of(aws, nt * NHD, [[1, NHD], [0, 4], [0, 4]])
                nc.vector.tensor_tensor(wflat, wflat, rcap, op=OP.mult)
            dbg("w_all", w_all[:], [128, NHD, NT, 16])

            # fold indices into wrapped layout [i%16, h*256 + (nt*4+p)*8 + ni//16]
            # step 1: DVE rearrange (nt,h,p)->(h,nt,p) within partitions
            rs = stile([128, NHD, NT, P], I16, "rs_idx")
            nc.vector.tensor_copy(
                rs[:], ap_of(ri16, 0, [[4, NHD], [32, NT], [1, 4]]))
            # step 2: per-nj DMA, contiguous src -> strided dst (3-dim APs)
            idxs = stile([128, NHD * 256], I16, "idxs")
            for nj in range(8):
                base = rs[nj * 16:(nj + 1) * 16, :, :, :]
                srci = bass.AP(tensor=base.tensor, offset=base.offset,
                               ap=[base.ap[0], [1, 256]])
                dfull = idxs[0:16, :]
                dst = bass.AP(tensor=dfull.tensor, offset=dfull.offset + nj,
                              ap=[dfull.ap[0], [256, NHD], [8, 32]])
                nc.sync.dma_start(dst, srci)
            for g in (16, 32, 64):
                nc.sync.dma_start(idxs[g:2 * g, :], idxs[0:g, :])
            dbg("idxs", idxs[:], [128, NHD * 256], I16)
            if "vcat0" in dbg_requested and "vcat0" not in dbg_exts:
                e = nc.declare_dram_parameter("dbg_vcat0", [GROWS, 128], BF16,
                                              isOutput=True)
                dbg_exts["vcat0"] = e
                nc.sync.dma_start(e.ap(), bass.AP(tensor=vcat[b].tensor, offset=vcat[b].offset,
                                                  ap=[[128, GROWS], [1, 128]]))

            st[b]["w_all"], st[b]["idxs"] = w_all, idxs

        def stage_G(b, st):
            w_all, idxs = st[b]["w_all"], st[b]["idxs"]
            vc = vcat[b]
            # gather + combine per h
            dfo = sp.tile([128, NT, C], BF16, name="dfo", tag="v2n", bufs=2)
            for h in range(NHD):
                G = stile([128, 32, 128], BF16, "G", bufs=3)
                if no_gather:
                    nc.vector.memset(G[:], 0.0)
                else:
                    nc.gpsimd.dma_gather(G[:],
                                         bass.AP(tensor=vc.tensor, offset=vc.offset,
                                                 ap=[[128, GROWS], [1, 128]]),
                                         idxs[:, h * 256:(h + 1) * 256], num_idxs=4096,
                                         num_idxs_reg=4096, elem_size=128,
                                         single_packet=False,
                                         queue_num=h % _nq)
                T = stile([128, 32, 4, 32], BF16, "T")
                nc.vector.tensor_tensor(
                    T[:], ap_of(G, 0, [[128, 32], [32, 4], [1, 32]]),
                    ap_of(w_all, h * NT * 16, [[4, 32], [1, 4], [0, 32]]), op=OP.mult)
                tc2 = sp.tile([128, 32, 2, 32], BF16, name="tc2", tag="tc2", bufs=2)
                nc.vector.tensor_tensor(tc2[:, :, 0, :], T[:, :, 0, :], T[:, :, 2, :],
                                        op=OP.add)
                nc.vector.tensor_tensor(tc2[:, :, 1, :], T[:, :, 1, :], T[:, :, 3, :],
                                        op=OP.add)
                t2 = sp.tile([128, 32, 32], BF16, name="t2", tag="T", bufs=1)
                nc.vector.tensor_tensor(t2[:], tc2[:, :, 0, :], tc2[:, :, 1, :], op=OP.add)
                o1 = stile([128, NT, 2, 32], BF16, "o1")
                nc.vector.tensor_tensor(o1[:], ap_of(t2, 0, [[128, NT], [32, 2], [1, 32]]),
                                        ap_of(t2, 64, [[128, NT], [32, 2], [1, 32]]),
                                        op=OP.add)
                nc.vector.tensor_tensor(ap_of(dfo, h * DH, [[C, NT], [1, DH]]),
                                        o1[:, :, 0, :], o1[:, :, 1, :],
                                        op=OP.add)
            dbg("dfo", dfo[:], [128, NT, C], BF16)

            st[b]["dfo"] = dfo

        def stage_C(b, st):
            dfo, x1 = st[b]["dfo"], st[b]["x1"]
            dfoT = stile([128, KC, N], BF16, "dfoT")
            for cb in range(KC):
                for n2 in range(NT // 2):
                    pt = ps_mm.tile([128, 256], BF16, name="pt_df", tag="pm")
                    for j in range(2):
                        nt = n2 * 2 + j
                        nc.tensor.transpose(pt[:, j * 128:(j + 1) * 128],
                                            dfo[:, nt, cb * 128:(cb + 1) * 128], ident[:])
                    nc.vector.tensor_copy(dfoT[:, cb, n2 * 256:(n2 + 1) * 256], pt[:])
            oprojT = sp.tile([128, KC, N], BF16, name="oprojT", tag="projT", bufs=2)
            matmul_f(dfoT, oproj_wT, C,
                     lambda pm, m, nch: nc.scalar.activation(
                         oprojT[:, m, nch * 512:(nch + 1) * 512], pm[:], AF.Identity,
                         bias=pcol[:, PC_OPROJB + m:PC_OPROJB + m + 1]))
            x2 = sp.tile([128, NT, C], F32, name="x2", tag="xres", bufs=4)
            transpose_back_add(oprojT, x1, x2)
            dbg("x2", x2[:], [128, NT, C])

            # ================= MLP =================
            ln3 = layernorm(x2, PC_LN3W, PC_LN3B, "ln3")
            hT = stile([128, HID // 128, N], BF16, "hT")
            matmul_f(ln3, fc1_wT, HID,
                     lambda pm, m, nch: nc.scalar.activation(
                         hT[:, m, nch * 512:(nch + 1) * 512], pm[:], AF.Gelu,
                         bias=pcol[:, PC_FC1B + m:PC_FC1B + m + 1]))
            f2T = sp.tile([128, KC, N], BF16, name="f2T", tag="projT", bufs=2)
            matmul_f(hT, fc2_wT, C,
                     lambda pm, m, nch: nc.scalar.activation(
                         f2T[:, m, nch * 512:(nch + 1) * 512], pm[:], AF.Identity,
                         bias=pcol[:, PC_FC2B + m:PC_FC2B + m + 1]),
                     kdim_tiles=HID // 128)
            x3 = sp.tile([128, NT, C], F32, name="x3", tag="xres", bufs=4)
            transpose_back_add(f2T, x2, x3)

            nc.sync.dma_start(out_ext.ap()[b].rearrange("(t p) c -> p t c", p=128), x3[:])

        def all_bodies():
            st = [{}, {}]
            stage_L(0, st)
            stage_L(1, st)
            stage_V(0, st)
            stage_A(0, st)
            stage_V(1, st)
            stage_P(0, st)
            stage_G(0, st)
            stage_A(1, st)
            stage_P(1, st)
            stage_C(0, st)
            stage_G(1, st)
            stage_C(1, st)

        if reps == 1:
            all_bodies()
        elif reps < 0:
            for rep in range(-reps):
                all_bodies()
        else:
            with tc.For_i(0, reps):
                all_bodies()

    nc.compile()
    return nc, dbg_exts


def host_prep(inputs, n_cores=8):
    """Preprocess FULL inputs -> list of per-core in_maps (B=16 -> 2 per core)."""
    import ml_dtypes
    bf16 = ml_dtypes.bfloat16
    f32 = np.float32
    g = {k: np.asarray(v) for k, v in inputs.items()}
    DHs = 1.0 / np.sqrt(DH)

    qkv_w = g["qkv_w"].astype(f32)                     # [3C, C] rows (qkv, h, d)
    qkv_wT = np.zeros((C, 768), f32)   # 6 chunks: q 0-1, k 2-3, v 4-5
    for t3 in range(3):
        for h in range(8):
            rows = qkv_w[t3 * C + h * DH: t3 * C + (h + 1) * DH]   # [32, C]
            col = (t3 * 2 + h // 4) * 128 + (h % 4) * 32
            qkv_wT[:, col:col + DH] = rows.T * (DHs if t3 == 0 else 1.0)
    # proj input features arrive as (h//4)*128 + (h%4)*32 + d
    perm = np.zeros(C, np.int64)
    for h in range(8):
        for d in range(DH):
            perm[(h // 4) * 128 + (h % 4) * 32 + d] = h * DH + d
    proj_wT = g["proj_w"].T.astype(f32)[perm]
    shared = {
        "qkv_wT": qkv_wT.astype(bf16),
        "proj_wT": proj_wT.astype(bf16).copy(),
        "vproj_wT": g["vproj_w"].T.astype(bf16).copy(),
        "oproj_wT": g["oproj_w"].T.astype(bf16).copy(),
        "offaw_wT": np.concatenate([g["off_w"], g["aw_w"]], 0).T.astype(bf16).copy(),
        "fc1_wT": g["fc1_w"].T.astype(bf16).copy(),
        "fc2_wT": g["fc2_w"].T.astype(bf16).copy(),
    }
    pcol = np.zeros((128, 28), f32)
    for col, v in ((PC_PROJB, g["proj_b"]), (PC_VPROJB, g["vproj_b"]),
                   (PC_OPROJB, g["oproj_b"]), (PC_FC1B, g["fc1_b"]),
                   (PC_FC2B, g["fc2_b"]), (PC_LN1W, g["ln1_w"]), (PC_LN1B, g["ln1_b"]),
                   (PC_LN2W, g["ln2_w"]), (PC_LN2B, g["ln2_b"]),
                   (PC_LN3W, g["ln3_w"]), (PC_LN3B, g["ln3_b"])):
        v = np.asarray(v, f32)
        pcol[:, col:col + v.size // 128] = v.reshape(-1, 128).T
    shared["pcol"] = pcol
    b96 = np.zeros((128, 96), f32)
    b96[:, 0:64] = g["off_b"].astype(f32) - 1.0        # -0.5 (grid) -0.5 (rint floor)
    b96[:, 64:96] = g["aw_b"].astype(f32)
    shared["b96"] = b96
    hr = np.zeros((128, 32), f32)
    # r = f*8 + h, f = y0*32 + x0 + 66 => r = y0*256 + x0*8 + 528 + h
    for h in range(NHD):
        for p in range(P):
            hr[:, h * 4 + p] = h + 528.0
    shared["hr"] = hr

    bpc = g["x"].shape[0] // n_cores
    assert bpc == B_L
    maps = []
    for c in range(n_cores):
        sl = slice(c * bpc, (c + 1) * bpc)
        m = dict(shared)
        m["x"] = g["x"][sl].astype(f32)
        m["ref32"] = (g["ref"][sl] * 32.0).astype(f32)
        m["valueT"] = np.ascontiguousarray(
            g["value"][sl].transpose(0, 2, 1)).astype(bf16)
        maps.append(m)
    return maps


_BUILD_CACHE = {}


def kernel(**inputs):
    """Full-input entry point: shards batch across 8 NeuronCores, runs the
    Bass kernel, gathers the full [16, 1024, 256] output."""
    from concourse.bass_utils import run_bass_kernel_spmd
    key = "k"
    if key not in _BUILD_CACHE:
        _BUILD_CACHE[key] = build(reps=1)[0]
    nc = _BUILD_CACHE[key]
    maps = host_prep(inputs, 8)
    res = run_bass_kernel_spmd(nc, maps, core_ids=list(range(8)))
    out = np.concatenate([res.results[c]["out"] for c in range(8)], axis=0)
    return out.astype(np.float32)
